# revision 21
# baseline (speedup 1.0000x reference)
"""Coverage-attention Trainium2 kernel (nn_Attention_44487271252662).

Data-parallel over batch across 8 NeuronCores (4 batches/core); weights
replicated. Per core the kernel streams enc_fea + enc_out (32 MB each)
once from HBM, so it is HBM-bandwidth bound (~190 us/core at ~360 GB/s).

Math per batch b (S=2048, F=1024):
  dec_fea = s_t @ dec_W.T + dec_b                      (PE, fp32r)
  att     = enc_fea + dec_fea[None,:] + coverage[:,None]*cov_w
            -- dec_fea/cov_w outer term via a K=2 PE matmul into PSUM,
               enc_fea added by a DVE scalar_tensor_tensor reading PSUM
  scores  = tanh(att) @ att_w                          (ACT tanh + DVE
            scalar_tensor_tensor with fused free-dim accumulate)
  attn    = exp(scores + ln(mask)) / sum(...)          (ACT exp w/ bias;
            cross-partition sum + 1/x broadcast via tiny PE matmuls)
  c_t     = attn @ enc_out                             (PE, fp32, PSUM
            accumulated over the 16 s-chunks, scaled by 1/sum at the end)
  cov_next = coverage + attn                           (DVE)

Host side only reshapes/transposes inputs (weight layout prep + batch
sharding) and concatenates the per-core outputs.
"""

import time

import numpy as np

B, S, F = 32, 2048, 1024
N_CORES = 8
BL = B // N_CORES        # batches per core
NCH = S // 128           # s-chunks per batch
FH = F // 2              # 512, one PSUM bank / max fp32 matmul N

_CACHE = {}


def _split_waits(nc, mybir, max_waits=1):
    # Walrus codegen in this env accepts at most one sync-wait command per
    # instruction; hoist extras onto same-engine NoOps placed just before.
    for f in nc.m.functions:
        for bb in f.blocks:
            new_insts = []
            for inst in bb.instructions:
                si = inst.sync_info
                if si is not None and si.on_wait and len(si.on_wait) > max_waits:
                    waits = list(si.on_wait)
                    for w in waits[max_waits:]:
                        new_insts.append(mybir.InstNoOp(
                            name=nc.get_next_instruction_name(),
                            engine=inst.engine, ins=[], outs=[],
                            sync_info=mybir.SyncInfo(on_wait=[w], on_update=[])))
                    inst.sync_info = mybir.SyncInfo(
                        on_wait=waits[:max_waits], on_update=list(si.on_update))
                new_insts.append(inst)
            bb.instructions = new_insts


def _build_nc(repeat=1):
    import concourse.bass as bass
    import concourse.mybir as mybir
    import concourse.tile as tile

    F32 = mybir.dt.float32
    F32R = mybir.dt.float32r
    AF = mybir.ActivationFunctionType
    ALU = mybir.AluOpType
    AX = mybir.AxisListType

    nc = bass.Bass()

    ef = nc.dram_tensor("ef", [BL * S, F], F32, kind="ExternalInput")
    eo = nc.dram_tensor("eo", [BL * S, F], F32, kind="ExternalInput")
    wtT = nc.dram_tensor("wtT", [F, F], F32, kind="ExternalInput")
    stT = nc.dram_tensor("stT", [F, BL], F32, kind="ExternalInput")
    decb = nc.dram_tensor("decb", [1, F], F32, kind="ExternalInput")
    ones14 = nc.dram_tensor("ones14", [1, BL], F32, kind="ExternalInput")
    covw = nc.dram_tensor("covw", [1, F], F32, kind="ExternalInput")
    attwb = nc.dram_tensor("attwb", [128, F], F32, kind="ExternalInput")
    cov_ones = nc.dram_tensor("cov_ones", [2 * BL, S], F32, kind="ExternalInput")
    cov_cols = nc.dram_tensor("cov_cols", [BL * 128, NCH], F32, kind="ExternalInput")
    mask_cols = nc.dram_tensor("mask_cols", [BL * 128, NCH], F32, kind="ExternalInput")
    ones_col = nc.dram_tensor("ones_col", [128, 1], F32, kind="ExternalInput")
    ones_row = nc.dram_tensor("ones_row", [1, 128], F32, kind="ExternalInput")

    o_ct = nc.dram_tensor("o_ct", [BL, F], F32, kind="ExternalOutput")
    o_ac = nc.dram_tensor("o_ac", [BL * 128, 2 * NCH], F32, kind="ExternalOutput")

    with tile.TileContext(nc) as tc:
        with (tc.tile_pool(name="const", bufs=1) as constp,
              tc.tile_pool(name="epool", bufs=7) as epool,
              tc.tile_pool(name="opool", bufs=7) as opool,
              tc.tile_pool(name="attp", bufs=2) as attp,
              tc.tile_pool(name="thp", bufs=2) as thp,
              tc.tile_pool(name="scp", bufs=4) as scp,
              tc.tile_pool(name="batchp", bufs=2) as batchp,
              tc.tile_pool(name="smallp", bufs=2) as smallp,
              tc.tile_pool(name="midp", bufs=1) as midp):

            # ------------- persistent constants -------------
            attw_sb = constp.tile([128, F], F32, tag="attw")
            onc_sb = constp.tile([128, 1], F32, tag="onc")
            onr_sb = constp.tile([1, 128], F32, tag="onr")
            # PE operands need 32-aligned partition bases, so each batch
            # gets its own [2, .] tile (f32r: rounded by the DVE cast that
            # writes it, as the fp32r matmul requires of its producers).
            covon_sb = [constp.tile([2, S], F32R, tag=f"covon{b}",
                                    name=f"covon{b}") for b in range(BL)]
            outer_rhs = [constp.tile([2, F], F32R, tag=f"orhs{b}",
                                     name=f"orhs{b}") for b in range(BL)]
            nc.sync.dma_start(attw_sb[:], attwb[:])
            nc.sync.dma_start(onc_sb[:], ones_col[:])
            nc.sync.dma_start(onr_sb[:], ones_row[:])

            dec_sb = midp.tile([BL, F], F32, tag="dec_sb")

            # ---- preamble: dec_fea = s_t @ W.T + b, f32r operand prep ----
            # covon/orhs raw tiles share one scoped pool (no later pool
            # reuses the range, avoiding overlap-dependencies on the casts)
            with tc.tile_pool(name="covp", bufs=1) as covp:
                covon_raw = [covp.tile([2, S], F32, tag="covonr", bufs=2,
                                       name=f"covonr{b}") for b in range(BL)]
                orhs_raw = [covp.tile([2, F], F32, tag="orhsr", bufs=2,
                                      name=f"orhsr{b}") for b in range(BL)]
                for b in range(BL):
                    nc.sync.dma_start(covon_raw[b][:],
                                      cov_ones[2 * b:2 * b + 2, :])
                # batch 0 cast now; batches 1-3 are deferred into chunk 0 so
                # they don't sit between the wt casts and the first STT
                nc.vector.tensor_copy(covon_sb[0][:], covon_raw[0][:])

                with (tc.tile_pool(name="phw", bufs=1) as phw,
                      tc.tile_pool(name="dec_ps", bufs=1, space="PSUM") as dec_ps):
                    stT_raw = phw.tile([128, 8 * BL], F32, tag="stT_raw")
                    stT_r = phw.tile([128, 8 * BL], F32R, tag="stT_r")
                    decb_raw = phw.tile([1, F], F32, tag="decb_raw")
                    decb_r = phw.tile([1, F], F32R, tag="decb_r")
                    on14_raw = phw.tile([1, BL], F32, tag="on14_raw")
                    on14_r = phw.tile([1, BL], F32R, tag="on14_r")
                    nc.sync.dma_start(decb_raw[:], decb[:])
                    nc.sync.dma_start(on14_raw[:], ones14[:])
                    nc.sync.dma_start(
                        stT_raw[:].rearrange("p (k b) -> p k b", b=BL),
                        stT[:].rearrange("(k p) b -> p k b", p=128))
                    nc.vector.tensor_copy(stT_r[:], stT_raw[:])
                    nc.vector.tensor_copy(decb_r[:], decb_raw[:])
                    nc.vector.tensor_copy(on14_r[:], on14_raw[:])
                    dec_psum = dec_ps.tile([BL, F], F32, tag="dec")
                    # 8 wt-chunk slots so every wt DMA is ready at schedule
                    # time (keeps them ahead of the enc prefetch on the SP
                    # queue); casts+matmuls consume rotating f32r slots.
                    # dec_b lands via K=1 ones matmuls closing the group.
                    for k in range(8):
                        wt_raw = phw.tile([128, F], F32, tag="wt_raw", bufs=7)
                        nc.sync.dma_start(wt_raw[:],
                                          wtT[k * 128:(k + 1) * 128, :])
                        wt_r = phw.tile([128, F], F32R, tag="wt_r", bufs=2)
                        nc.vector.tensor_copy(wt_r[:], wt_raw[:])
                        for h in range(2):
                            nc.tensor.matmul(
                                dec_psum[:, h * FH:(h + 1) * FH],
                                stT_r[:, k * BL:(k + 1) * BL],
                                wt_r[:, h * FH:(h + 1) * FH],
                                start=(k == 0), stop=False)
                    for h in range(2):
                        nc.tensor.matmul(
                            dec_psum[:, h * FH:(h + 1) * FH],
                            on14_r[:],
                            decb_r[:, h * FH:(h + 1) * FH],
                            start=False, stop=True)
                    nc.scalar.copy(dec_sb[:], dec_psum[:])

                for b in range(BL):
                    # rows: 0 = cov_w (DRAM), 1 = dec_fea[b] (SBUF->SBUF).
                    # gpsimd queue keeps the SP queue free for the big
                    # stream; ACT does the f32r rounding cast.
                    nc.gpsimd.dma_start(orhs_raw[b][0:1, :], covw[:])
                    nc.gpsimd.dma_start(orhs_raw[b][1:2, :], dec_sb[b:b + 1, :])
                    nc.scalar.copy(outer_rhs[b][:], orhs_raw[b][:])
                for b in range(1, BL):
                    nc.scalar.copy(covon_sb[b][:], covon_raw[b][:])

            # ------------- streaming main loop -------------
            with (tc.tile_pool(name="outer_ps", bufs=2, space="PSUM") as outer_ps,
                  tc.tile_pool(name="ct_ps", bufs=1, space="PSUM") as ct_ps,
                  tc.tile_pool(name="tot_ps", bufs=2, space="PSUM") as tot_ps):

                for rep in range(repeat):
                  for b in range(BL):
                      exp_t = batchp.tile([128, NCH], F32, tag="exp")
                      mask_t = batchp.tile([128, NCH], F32, tag="mask")
                      lm_t = batchp.tile([128, NCH], F32, tag="lm")
                      covc_t = batchp.tile([128, NCH], F32, tag="covc")
                      nc.gpsimd.dma_start(mask_t[:],
                                          mask_cols[b * 128:(b + 1) * 128, :])
                      nc.gpsimd.dma_start(covc_t[:],
                                          cov_cols[b * 128:(b + 1) * 128, :])
                      nc.scalar.activation(lm_t[:], mask_t[:], AF.Ln)

                      ct_psum = ct_ps.tile([1, F], F32, tag="ct")
                      ptotbc = tot_ps.tile([128, 2], F32, tag="ptotbc")

                      for i in range(NCH):
                          row0 = (b * S + i * 128)
                          e_t = epool.tile([128, F], F32, tag="e")
                          o_t = opool.tile([128, F], F32, tag="o")
                          nc.sync.dma_start(e_t[:], ef[row0:row0 + 128, :])
                          nc.sync.dma_start(o_t[:], eo[row0:row0 + 128, :])

                          # att = enc_fea + (coverage*cov_w + dec_fea) ; tanh
                          p_out = outer_ps.tile([128, F], F32, tag="outer")
                          for h in range(2):
                              nc.tensor.matmul(
                                  p_out[:, h * FH:(h + 1) * FH],
                                  covon_sb[b][:, i * 128:(i + 1) * 128],
                                  outer_rhs[b][:, h * FH:(h + 1) * FH],
                                  start=True, stop=True)
                          att_t = attp.tile([128, F], F32, tag="att")
                          nc.vector.scalar_tensor_tensor(
                              out=att_t[:], in0=e_t[:], scalar=0.0, in1=p_out[:],
                              op0=ALU.add, op1=ALU.add)
                          th_t = thp.tile([128, F], F32, tag="th")
                          nc.scalar.activation(th_t[:], att_t[:], AF.Tanh)

                          # scores (fused *att_w + free-dim accumulate), exp
                          sc_t = scp.tile([128, 1], F32, tag="sc")
                          nc.vector.scalar_tensor_tensor(
                              out=att_t[:], in0=th_t[:], scalar=1.0,
                              in1=attw_sb[:], op0=ALU.mult, op1=ALU.mult,
                              accum_out=sc_t[:])
                          nc.scalar.activation(exp_t[:, i:i + 1], sc_t[:],
                                               AF.Exp, bias=lm_t[:, i:i + 1],
                                               scale=1.0)

                          # c_t += exp.T @ enc_out (unnormalized)
                          for h in range(2):
                              nc.tensor.matmul(
                                  ct_psum[0:1, h * FH:(h + 1) * FH],
                                  exp_t[:, i:i + 1],
                                  o_t[:, h * FH:(h + 1) * FH],
                                  start=(i == 0), stop=(i == NCH - 1))
                          # running softmax total on the PE (frees the
                          # epilogue from the sums->total chain)
                          nc.tensor.matmul(
                              ptotbc[0:1, 0:1], exp_t[:, i:i + 1],
                              onc_sb[:], start=(i == 0), stop=(i == NCH - 1))

                      # ------------- batch epilogue -------------
                      recip_t = smallp.tile([1, 1], F32, tag="recip")
                      nc.vector.reciprocal(recip_t[:], ptotbc[0:1, 0:1])
                      nc.tensor.matmul(ptotbc[:, 1:2], onr_sb[:], recip_t[:],
                                       start=True, stop=True)

                      ac_t = smallp.tile([128, 2 * NCH], F32, tag="ac")
                      nc.vector.tensor_scalar_mul(ac_t[:, 0:NCH], exp_t[:],
                                                  ptotbc[:, 1:2])
                      nc.vector.tensor_tensor(out=ac_t[:, NCH:2 * NCH],
                                              in0=ac_t[:, 0:NCH],
                                              in1=covc_t[:], op=ALU.add)
                      ct_row = smallp.tile([1, F], F32, tag="ct_row")
                      nc.vector.tensor_scalar_mul(ct_row[:], ct_psum[:],
                                                  recip_t[:])
                      nc.gpsimd.dma_start(o_ac[b * 128:(b + 1) * 128, :],
                                          ac_t[:])
                      nc.gpsimd.dma_start(o_ct[b:b + 1, :], ct_row[:])

    import concourse.mybir as mybir2
    _split_waits(nc, mybir2)
    return nc


def _prep_inputs(s_t, enc_out, enc_fea, enc_pad_mask, coverage,
                 dec_W, dec_b, att_w, cov_w):
    s_t = np.asarray(s_t, np.float32)
    enc_out = np.asarray(enc_out, np.float32)
    enc_fea = np.asarray(enc_fea, np.float32)
    enc_pad_mask = np.asarray(enc_pad_mask, np.float32)
    coverage = np.asarray(coverage, np.float32)
    dec_W = np.asarray(dec_W, np.float32)
    dec_b = np.asarray(dec_b, np.float32)
    att_w = np.asarray(att_w, np.float32)
    cov_w = np.asarray(cov_w, np.float32)

    wtT = np.ascontiguousarray(dec_W.T)
    decb = dec_b[None, :]
    ones14 = np.ones((1, BL), np.float32)
    covw = cov_w[None, :]
    attwb = np.ascontiguousarray(np.broadcast_to(att_w, (128, F)))
    onc = np.ones((128, 1), np.float32)
    onr = np.ones((1, 128), np.float32)

    in_maps = []
    for c in range(N_CORES):
        b0, b1 = c * BL, (c + 1) * BL
        cov_sh = coverage[b0:b1]
        mask_sh = enc_pad_mask[b0:b1]
        cov_ones = np.empty((2 * BL, S), np.float32)
        cov_ones[0::2] = cov_sh
        cov_ones[1::2] = 1.0
        in_maps.append({
            "ef": np.ascontiguousarray(enc_fea[b0 * S:b1 * S]),
            "eo": np.ascontiguousarray(enc_out[b0:b1].reshape(BL * S, F)),
            "wtT": wtT,
            "stT": np.ascontiguousarray(s_t[b0:b1].T),
            "decb": decb,
            "ones14": ones14,
            "covw": covw,
            "attwb": attwb,
            "cov_ones": cov_ones,
            "cov_cols": np.ascontiguousarray(
                cov_sh.reshape(BL, NCH, 128).transpose(0, 2, 1).reshape(BL * 128, NCH)),
            "mask_cols": np.ascontiguousarray(
                mask_sh.reshape(BL, NCH, 128).transpose(0, 2, 1).reshape(BL * 128, NCH)),
            "ones_col": onc,
            "ones_row": onr,
        })
    return in_maps


def _assemble(results):
    c_t = np.concatenate([r["o_ct"] for r in results], axis=0)
    attn = np.concatenate(
        [r["o_ac"][:, 0:NCH].reshape(BL, 128, NCH).transpose(0, 2, 1)
         .reshape(BL, S) for r in results], axis=0)
    covn = np.concatenate(
        [r["o_ac"][:, NCH:2 * NCH].reshape(BL, 128, NCH).transpose(0, 2, 1)
         .reshape(BL, S) for r in results], axis=0)
    return c_t, attn, covn


def kernel(s_t, enc_out, enc_fea, enc_pad_mask, coverage,
           dec_W, dec_b, att_w, cov_w):
    from concourse.bass_utils import run_bass_kernel_spmd

    if "nc" not in _CACHE:
        _CACHE["nc"] = _build_nc()
    nc = _CACHE["nc"]

    in_maps = _prep_inputs(s_t, enc_out, enc_fea, enc_pad_mask, coverage,
                           dec_W, dec_b, att_w, cov_w)
    try:
        res = run_bass_kernel_spmd(nc, in_maps, list(range(N_CORES)))
    except Exception:
        # transient NRT/device hiccups have been observed once; retry once
        time.sleep(2)
        res = run_bass_kernel_spmd(nc, in_maps, list(range(N_CORES)))
    return _assemble(res.results)



# revision 26
# speedup vs baseline: 1.0083x; 1.0083x over previous
"""Coverage-attention Trainium2 kernel (nn_Attention_44487271252662).

Data-parallel over batch across 8 NeuronCores (4 batches/core); weights
replicated. Per core the kernel streams enc_fea + enc_out (32 MB each)
once from HBM, so it is HBM-bandwidth bound (~190 us/core at ~360 GB/s).

Math per batch b (S=2048, F=1024):
  dec_fea = s_t @ dec_W.T + dec_b                      (PE, fp32r)
  att     = enc_fea + dec_fea[None,:] + coverage[:,None]*cov_w
            -- dec_fea/cov_w outer term via a K=2 PE matmul into PSUM,
               enc_fea added by a DVE scalar_tensor_tensor reading PSUM
  scores  = tanh(att) @ att_w                          (ACT tanh + DVE
            scalar_tensor_tensor with fused free-dim accumulate)
  attn    = exp(scores + ln(mask)) / sum(...)          (ACT exp w/ bias;
            cross-partition sum + 1/x broadcast via tiny PE matmuls)
  c_t     = attn @ enc_out                             (PE, fp32, PSUM
            accumulated over the 16 s-chunks, scaled by 1/sum at the end)
  cov_next = coverage + attn                           (DVE)

Host side only reshapes/transposes inputs (weight layout prep + batch
sharding) and concatenates the per-core outputs.
"""

import time

import numpy as np

B, S, F = 32, 2048, 1024
N_CORES = 8
BL = B // N_CORES        # batches per core
NCH = S // 128           # s-chunks per batch
FH = F // 2              # 512, one PSUM bank / max fp32 matmul N

_CACHE = {}


def _split_waits(nc, mybir, max_waits=1):
    # Walrus codegen in this env accepts at most one sync-wait command per
    # instruction; hoist extras onto same-engine NoOps placed just before.
    for f in nc.m.functions:
        for bb in f.blocks:
            new_insts = []
            for inst in bb.instructions:
                si = inst.sync_info
                if si is not None and si.on_wait and len(si.on_wait) > max_waits:
                    waits = list(si.on_wait)
                    for w in waits[max_waits:]:
                        new_insts.append(mybir.InstNoOp(
                            name=nc.get_next_instruction_name(),
                            engine=inst.engine, ins=[], outs=[],
                            sync_info=mybir.SyncInfo(on_wait=[w], on_update=[])))
                    inst.sync_info = mybir.SyncInfo(
                        on_wait=waits[:max_waits], on_update=list(si.on_update))
                new_insts.append(inst)
            bb.instructions = new_insts


def _build_nc(repeat=1):
    import concourse.bass as bass
    import concourse.mybir as mybir
    import concourse.tile as tile

    F32 = mybir.dt.float32
    F16 = mybir.dt.float16
    F32R = mybir.dt.float32r
    AF = mybir.ActivationFunctionType
    ALU = mybir.AluOpType
    AX = mybir.AxisListType

    nc = bass.Bass()

    ef = nc.dram_tensor("ef", [BL * S, F], F32, kind="ExternalInput")
    eo = nc.dram_tensor("eo", [BL * S, F], F32, kind="ExternalInput")
    wtT = nc.dram_tensor("wtT", [F, F], F32, kind="ExternalInput")
    stT = nc.dram_tensor("stT", [F, BL], F32, kind="ExternalInput")
    decb = nc.dram_tensor("decb", [1, F], F32, kind="ExternalInput")
    ones14 = nc.dram_tensor("ones14", [1, BL], F32, kind="ExternalInput")
    covw = nc.dram_tensor("covw", [1, F], F32, kind="ExternalInput")
    attwb = nc.dram_tensor("attwb", [128, F], F16, kind="ExternalInput")
    cov_ones = nc.dram_tensor("cov_ones", [2 * BL, S], F32, kind="ExternalInput")
    cov_cols = nc.dram_tensor("cov_cols", [BL * 128, NCH], F32, kind="ExternalInput")
    mask_cols = nc.dram_tensor("mask_cols", [BL * 128, NCH], F32, kind="ExternalInput")
    ones_col = nc.dram_tensor("ones_col", [128, 1], F32, kind="ExternalInput")
    ones_row = nc.dram_tensor("ones_row", [1, 128], F32, kind="ExternalInput")

    o_ct = nc.dram_tensor("o_ct", [BL, F], F32, kind="ExternalOutput")
    o_ac = nc.dram_tensor("o_ac", [BL * 128, 2 * NCH], F32, kind="ExternalOutput")

    with tile.TileContext(nc) as tc:
        with (tc.tile_pool(name="const", bufs=1) as constp,
              tc.tile_pool(name="epool", bufs=7) as epool,
              tc.tile_pool(name="opool", bufs=7) as opool,
              tc.tile_pool(name="attp", bufs=2) as attp,
              tc.tile_pool(name="thp", bufs=2) as thp,
              tc.tile_pool(name="scp", bufs=4) as scp,
              tc.tile_pool(name="batchp", bufs=2) as batchp,
              tc.tile_pool(name="smallp", bufs=2) as smallp,
              tc.tile_pool(name="midp", bufs=1) as midp):

            # ------------- persistent constants -------------
            attw_sb = constp.tile([128, F], F16, tag="attw")
            onc_sb = constp.tile([128, 1], F32, tag="onc")
            onr_sb = constp.tile([1, 128], F32, tag="onr")
            # PE operands need 32-aligned partition bases, so each batch
            # gets its own [2, .] tile (f32r: rounded by the DVE cast that
            # writes it, as the fp32r matmul requires of its producers).
            covon_sb = [constp.tile([2, S], F32R, tag=f"covon{b}",
                                    name=f"covon{b}") for b in range(BL)]
            outer_rhs = [constp.tile([2, F], F32R, tag=f"orhs{b}",
                                     name=f"orhs{b}") for b in range(BL)]
            nc.sync.dma_start(attw_sb[:], attwb[:])
            nc.sync.dma_start(onc_sb[:], ones_col[:])
            nc.sync.dma_start(onr_sb[:], ones_row[:])

            dec_sb = midp.tile([BL, F], F32, tag="dec_sb")

            # ---- preamble: dec_fea = s_t @ W.T + b, f32r operand prep ----
            # covon/orhs raw tiles share one scoped pool (no later pool
            # reuses the range, avoiding overlap-dependencies on the casts)
            with tc.tile_pool(name="covp", bufs=1) as covp:
                covon_raw = [covp.tile([2, S], F32, tag="covonr", bufs=2,
                                       name=f"covonr{b}") for b in range(BL)]
                orhs_raw = [covp.tile([2, F], F32, tag="orhsr", bufs=2,
                                      name=f"orhsr{b}") for b in range(BL)]
                for b in range(BL):
                    nc.sync.dma_start(covon_raw[b][:],
                                      cov_ones[2 * b:2 * b + 2, :])
                # batch 0 cast now; batches 1-3 are deferred into chunk 0 so
                # they don't sit between the wt casts and the first STT
                nc.vector.tensor_copy(covon_sb[0][:], covon_raw[0][:])

                with (tc.tile_pool(name="phw", bufs=1) as phw,
                      tc.tile_pool(name="dec_ps", bufs=1, space="PSUM") as dec_ps):
                    stT_raw = phw.tile([128, 8 * BL], F32, tag="stT_raw")
                    stT_r = phw.tile([128, 8 * BL], F32R, tag="stT_r")
                    decb_raw = phw.tile([1, F], F32, tag="decb_raw")
                    decb_r = phw.tile([1, F], F32R, tag="decb_r")
                    on14_raw = phw.tile([1, BL], F32, tag="on14_raw")
                    on14_r = phw.tile([1, BL], F32R, tag="on14_r")
                    nc.sync.dma_start(decb_raw[:], decb[:])
                    nc.sync.dma_start(on14_raw[:], ones14[:])
                    nc.sync.dma_start(
                        stT_raw[:].rearrange("p (k b) -> p k b", b=BL),
                        stT[:].rearrange("(k p) b -> p k b", p=128))
                    nc.vector.tensor_copy(stT_r[:], stT_raw[:])
                    nc.vector.tensor_copy(decb_r[:], decb_raw[:])
                    nc.vector.tensor_copy(on14_r[:], on14_raw[:])
                    dec_psum = dec_ps.tile([BL, F], F32, tag="dec")
                    # 8 wt-chunk slots so every wt DMA is ready at schedule
                    # time (keeps them ahead of the enc prefetch on the SP
                    # queue); casts+matmuls consume rotating f32r slots.
                    # dec_b lands via K=1 ones matmuls closing the group.
                    for k in range(8):
                        wt_raw = phw.tile([128, F], F32, tag="wt_raw", bufs=7)
                        nc.sync.dma_start(wt_raw[:],
                                          wtT[k * 128:(k + 1) * 128, :])
                        wt_r = phw.tile([128, F], F32R, tag="wt_r", bufs=2)
                        nc.vector.tensor_copy(wt_r[:], wt_raw[:])
                        for h in range(2):
                            nc.tensor.matmul(
                                dec_psum[:, h * FH:(h + 1) * FH],
                                stT_r[:, k * BL:(k + 1) * BL],
                                wt_r[:, h * FH:(h + 1) * FH],
                                start=(k == 0), stop=False)
                    for h in range(2):
                        nc.tensor.matmul(
                            dec_psum[:, h * FH:(h + 1) * FH],
                            on14_r[:],
                            decb_r[:, h * FH:(h + 1) * FH],
                            start=False, stop=True)
                    nc.scalar.copy(dec_sb[:], dec_psum[:])

                for b in range(BL):
                    # rows: 0 = cov_w (DRAM), 1 = dec_fea[b] (SBUF->SBUF).
                    # gpsimd queue keeps the SP queue free for the big
                    # stream; ACT does the f32r rounding cast.
                    nc.gpsimd.dma_start(orhs_raw[b][0:1, :], covw[:])
                    nc.gpsimd.dma_start(orhs_raw[b][1:2, :], dec_sb[b:b + 1, :])
                    nc.scalar.copy(outer_rhs[b][:], orhs_raw[b][:])
                for b in range(1, BL):
                    nc.scalar.copy(covon_sb[b][:], covon_raw[b][:])

            # ------------- streaming main loop -------------
            with (tc.tile_pool(name="outer_ps", bufs=2, space="PSUM") as outer_ps,
                  tc.tile_pool(name="ct_ps", bufs=1, space="PSUM") as ct_ps,
                  tc.tile_pool(name="tot_ps", bufs=2, space="PSUM") as tot_ps):

                for rep in range(repeat):
                  for b in range(BL):
                      exp_t = batchp.tile([128, NCH], F32, tag="exp")
                      mask_t = batchp.tile([128, NCH], F32, tag="mask")
                      lm_t = batchp.tile([128, NCH], F32, tag="lm")
                      covc_t = batchp.tile([128, NCH], F32, tag="covc")
                      nc.gpsimd.dma_start(mask_t[:],
                                          mask_cols[b * 128:(b + 1) * 128, :])
                      nc.gpsimd.dma_start(covc_t[:],
                                          cov_cols[b * 128:(b + 1) * 128, :])
                      nc.scalar.activation(lm_t[:], mask_t[:], AF.Ln)

                      ct_psum = ct_ps.tile([1, F], F32, tag="ct")
                      ptotbc = tot_ps.tile([128, 2], F32, tag="ptotbc")

                      for i in range(NCH):
                          row0 = (b * S + i * 128)
                          e_t = epool.tile([128, F], F32, tag="e")
                          o_t = opool.tile([128, F], F32, tag="o")
                          nc.sync.dma_start(e_t[:], ef[row0:row0 + 128, :])
                          nc.sync.dma_start(o_t[:], eo[row0:row0 + 128, :])

                          # att = enc_fea + (coverage*cov_w + dec_fea) ; tanh
                          p_out = outer_ps.tile([128, F], F32, tag="outer")
                          for h in range(2):
                              nc.tensor.matmul(
                                  p_out[:, h * FH:(h + 1) * FH],
                                  covon_sb[b][:, i * 128:(i + 1) * 128],
                                  outer_rhs[b][:, h * FH:(h + 1) * FH],
                                  start=True, stop=True)
                          att_t = attp.tile([128, F], F32, tag="att")
                          nc.vector.scalar_tensor_tensor(
                              out=att_t[:], in0=e_t[:], scalar=0.0, in1=p_out[:],
                              op0=ALU.add, op1=ALU.add)
                          # tanh in fp16: the scores pass then runs in the
                          # DVE 2x perf mode (all 2-byte operands); products
                          # and the accumulate stay fp32 inside the DVE
                          th_t = thp.tile([128, F], F16, tag="th")
                          nc.scalar.activation(th_t[:], att_t[:], AF.Tanh)

                          # scores (fused *att_w + free-dim accumulate), exp
                          sc_t = scp.tile([128, 1], F32, tag="sc")
                          th2_t = thp.tile([128, F], F16, tag="th2")
                          nc.vector.scalar_tensor_tensor(
                              out=th2_t[:], in0=th_t[:], scalar=1.0,
                              in1=attw_sb[:], op0=ALU.mult, op1=ALU.mult,
                              accum_out=sc_t[:])
                          nc.scalar.activation(exp_t[:, i:i + 1], sc_t[:],
                                               AF.Exp, bias=lm_t[:, i:i + 1],
                                               scale=1.0)

                          # c_t += exp.T @ enc_out (unnormalized)
                          for h in range(2):
                              nc.tensor.matmul(
                                  ct_psum[0:1, h * FH:(h + 1) * FH],
                                  exp_t[:, i:i + 1],
                                  o_t[:, h * FH:(h + 1) * FH],
                                  start=(i == 0), stop=(i == NCH - 1))
                          # running softmax total on the PE (frees the
                          # epilogue from the sums->total chain)
                          nc.tensor.matmul(
                              ptotbc[0:1, 0:1], exp_t[:, i:i + 1],
                              onc_sb[:], start=(i == 0), stop=(i == NCH - 1))

                      # ------------- batch epilogue -------------
                      recip_t = smallp.tile([1, 1], F32, tag="recip")
                      nc.vector.reciprocal(recip_t[:], ptotbc[0:1, 0:1])
                      nc.tensor.matmul(ptotbc[:, 1:2], onr_sb[:], recip_t[:],
                                       start=True, stop=True)

                      ac_t = smallp.tile([128, 2 * NCH], F32, tag="ac")
                      nc.vector.tensor_scalar_mul(ac_t[:, 0:NCH], exp_t[:],
                                                  ptotbc[:, 1:2])
                      nc.vector.tensor_tensor(out=ac_t[:, NCH:2 * NCH],
                                              in0=ac_t[:, 0:NCH],
                                              in1=covc_t[:], op=ALU.add)
                      ct_row = smallp.tile([1, F], F32, tag="ct_row")
                      nc.vector.tensor_scalar_mul(ct_row[:], ct_psum[:],
                                                  recip_t[:])
                      nc.gpsimd.dma_start(o_ac[b * 128:(b + 1) * 128, :],
                                          ac_t[:])
                      nc.gpsimd.dma_start(o_ct[b:b + 1, :], ct_row[:])

    import concourse.mybir as mybir2
    _split_waits(nc, mybir2)
    return nc


def _prep_inputs(s_t, enc_out, enc_fea, enc_pad_mask, coverage,
                 dec_W, dec_b, att_w, cov_w):
    s_t = np.asarray(s_t, np.float32)
    enc_out = np.asarray(enc_out, np.float32)
    enc_fea = np.asarray(enc_fea, np.float32)
    enc_pad_mask = np.asarray(enc_pad_mask, np.float32)
    coverage = np.asarray(coverage, np.float32)
    dec_W = np.asarray(dec_W, np.float32)
    dec_b = np.asarray(dec_b, np.float32)
    att_w = np.asarray(att_w, np.float32)
    cov_w = np.asarray(cov_w, np.float32)

    wtT = np.ascontiguousarray(dec_W.T)
    decb = dec_b[None, :]
    ones14 = np.ones((1, BL), np.float32)
    covw = cov_w[None, :]
    attwb = np.ascontiguousarray(
        np.broadcast_to(att_w.astype(np.float16), (128, F)))
    onc = np.ones((128, 1), np.float32)
    onr = np.ones((1, 128), np.float32)

    in_maps = []
    for c in range(N_CORES):
        b0, b1 = c * BL, (c + 1) * BL
        cov_sh = coverage[b0:b1]
        mask_sh = enc_pad_mask[b0:b1]
        cov_ones = np.empty((2 * BL, S), np.float32)
        cov_ones[0::2] = cov_sh
        cov_ones[1::2] = 1.0
        in_maps.append({
            "ef": np.ascontiguousarray(enc_fea[b0 * S:b1 * S]),
            "eo": np.ascontiguousarray(enc_out[b0:b1].reshape(BL * S, F)),
            "wtT": wtT,
            "stT": np.ascontiguousarray(s_t[b0:b1].T),
            "decb": decb,
            "ones14": ones14,
            "covw": covw,
            "attwb": attwb,
            "cov_ones": cov_ones,
            "cov_cols": np.ascontiguousarray(
                cov_sh.reshape(BL, NCH, 128).transpose(0, 2, 1).reshape(BL * 128, NCH)),
            "mask_cols": np.ascontiguousarray(
                mask_sh.reshape(BL, NCH, 128).transpose(0, 2, 1).reshape(BL * 128, NCH)),
            "ones_col": onc,
            "ones_row": onr,
        })
    return in_maps


def _assemble(results):
    c_t = np.concatenate([r["o_ct"] for r in results], axis=0)
    attn = np.concatenate(
        [r["o_ac"][:, 0:NCH].reshape(BL, 128, NCH).transpose(0, 2, 1)
         .reshape(BL, S) for r in results], axis=0)
    covn = np.concatenate(
        [r["o_ac"][:, NCH:2 * NCH].reshape(BL, 128, NCH).transpose(0, 2, 1)
         .reshape(BL, S) for r in results], axis=0)
    return c_t, attn, covn


def kernel(s_t, enc_out, enc_fea, enc_pad_mask, coverage,
           dec_W, dec_b, att_w, cov_w):
    from concourse.bass_utils import run_bass_kernel_spmd

    if "nc" not in _CACHE:
        _CACHE["nc"] = _build_nc()
    nc = _CACHE["nc"]

    in_maps = _prep_inputs(s_t, enc_out, enc_fea, enc_pad_mask, coverage,
                           dec_W, dec_b, att_w, cov_w)
    try:
        res = run_bass_kernel_spmd(nc, in_maps, list(range(N_CORES)))
    except Exception:
        # transient NRT/device hiccups have been observed once; retry once
        time.sleep(2)
        res = run_bass_kernel_spmd(nc, in_maps, list(range(N_CORES)))
    return _assemble(res.results)

